# revision 7
# baseline (speedup 1.0000x reference)
"""Trainium2 Bass kernel for zero-phase Butterworth band-stop filter (filtfilt).

Single fused pass: both filtfilt IIR sweeps collapse into one banded
block-Toeplitz convolution with the symmetric autocorrelation kernel
g = h (*) h_rev of the filter impulse response h:

    y[m] = sum_{j=-J..J} F_j @ u[m+j]    (F_j[i,p] = g[i - p - 128 j])

plus two small boundary terms (all matrices host-built in float64):
  * left:  zi transient of pass 1, rank-1 per lane in x0 = ext[Z0]
           (outer-product matmuls with contraction dim 1)
  * right: pass-2 right-edge correction D @ s, where s is the 16-dim
           state (last-8 y1, last-8 u); computed in full fp32 because D
           has ~70x non-normal cancellation. y1's last 8 samples come
           from 3 small fp32 matmuls against unrounded input tails.

All full-width matmuls run in float32r (host-prerounded inputs), which
streams at 1 column/cycle on the PE instead of fp32's 4. Data layout is
block-major ([128 rows = in-block position] x [col = block*4 + lane])
with 8 zero-pad columns on each side, so the shifted operands of F_j are
plain column-offset views of one SBUF tile - no stack DMAs at all.
Output is shipped as bf16 (upcast on host): rounding adds ~2e-3 relmax,
10% of the tolerance, and halves the output DMA bytes.

Sharding: 32 lanes (batch*channel), 4 per NeuronCore across 8 cores.
"""
import os

import numpy as np

import concourse.bacc as bacc
import concourse.mybir as mybir
import concourse.tile as tile
from concourse.bass_utils import run_bass_kernel_spmd

# ---------------- problem geometry (hardcoded for this problem) ----------------
BSH, CSH, T = 4, 8, 131072
LANES = BSH * CSH               # 32
N_CORES = 8
LPC = LANES // N_CORES          # 4 lanes per core
PADLEN = 27
BLK = 128
Z0 = 74                          # front zero padding so ext ends on block edge
L = Z0 + T + 2 * PADLEN          # 131200 samples per lane
NB = L // BLK                    # 1025 blocks per lane
CR = LPC * NB                    # 4100 real columns per core
PF = 8                           # front zero-pad cols (2 blocks)
PB = 8                           # back zero-pad cols
UCOLS = PF + CR + PB             # 4116
NS = 10                          # matmul strips
SW = CR // NS                    # 410 cols per strip (psum bank = 512 f32 max)
JUSE = 1                         # F_j for j in [-JUSE, JUSE]
NF = 2 * JUSE + 1
LH = 640                         # impulse-response length kept
WLB = 2                          # left-zi blocks corrected
DBLK = 3                         # right-edge blocks corrected
NO = 8                           # filter order
OUT_BF16 = True

F32 = mybir.dt.float32
F32R = mybir.dt.float32r
BF16 = mybir.dt.bfloat16
ODT = BF16 if OUT_BF16 else F32

# blob column layout (f32r dram [128, BLOBC])
WF_OFF = 0                       # F lhsT   [128, NF*128] f32r
UH_OFF = WF_OFF + NF * BLK       # U3 [128,12] + HT [128,24] f32 (bitcast)
D_OFF = UH_OFF + 36              # D lhsT rows 0:16 [16, DBLK*128] f32
#   + utail at rows 8:16, cols D_OFF+DBLK*128 : +4 (f32)
DS_COLS = DBLK * BLK + 4
WX_OFF = D_OFF + DS_COLS         # wl lhsT rows 0:1 [1, WLB*128] + x0 [1,4] f32r
WX_COLS = WLB * BLK + 4
UOFF = WX_OFF + WX_COLS          # U region [128, UCOLS] f32r (pads zeroed)
BLOBC = UOFF + UCOLS

_matrix_cache: dict = {}
_nc_cache: dict = {}
last_exec_time_ns = None


# ---------------- host-side matrix construction (float64) ----------------
def _round_f32r(v):
    """fp32r pre-rounding: bf16 hi + bf16 lo split (matches device cast)."""
    def bf16(x):
        u = np.ascontiguousarray(x, dtype=np.float32).view(np.uint32)
        return (((u + 0x7FFF + ((u >> 16) & 1)) & 0xFFFF0000)
                .astype(np.uint32)).view(np.float32)
    v32 = np.asarray(v, dtype=np.float32)
    hi = bf16(v32)
    lo = bf16((v32.astype(np.float64) - hi.astype(np.float64)).astype(np.float32))
    return (hi.astype(np.float64) + lo.astype(np.float64)).astype(np.float32)


def _build_matrices(b64, a64):
    key = (b64.tobytes(), a64.tobytes())
    if key in _matrix_cache:
        return _matrix_cache[key]
    bh = b64 / a64[0]
    ah = a64 / a64[0]

    def lfilter1(x):
        y = np.empty_like(x)
        z = np.zeros(NO)
        for t in range(x.shape[0]):
            xt = x[t]
            yt = bh[0] * xt + z[0]
            z[:-1] = z[1:]
            z[-1] = 0.0
            z += bh[1:] * xt - ah[1:] * yt
            y[t] = yt
        return y

    def ar_resp(drive):
        y = np.zeros(drive.shape[0])
        for t in range(y.shape[0]):
            v = drive[t]
            for k in range(1, NO + 1):
                if t - k >= 0:
                    v -= ah[k] * y[t - k]
            y[t] = v
        return y

    imp = np.zeros(LH)
    imp[0] = 1.0
    h = lfilter1(imp)
    g = np.correlate(h, h, mode="full")
    g0 = LH - 1

    ii = np.arange(BLK)[:, None]
    pp = np.arange(BLK)[None, :]
    Fts = []
    for j in range(-JUSE, JUSE + 1):
        d = ii - pp - BLK * j
        Fj = np.zeros((BLK, BLK))
        mask = np.abs(d) <= (LH - 1)
        Fj[mask] = g[d[mask] + g0]
        Fts.append(Fj.T.copy())

    A = np.zeros((NO, NO))
    A[0] = -ah[1:]
    A[np.arange(1, NO), np.arange(0, NO - 1)] = 1.0
    zi = np.linalg.solve(np.eye(NO) - A.T, bh[1:] - ah[1:] * bh[0])

    # left correction: zi transient of pass 1 through anticausal pass 2
    LT = WLB * BLK
    drive = np.zeros(LT + LH)
    drive[Z0:Z0 + NO] = zi
    t1 = ar_resp(drive)
    wl = np.zeros(LT)
    for t in range(LT):
        wl[t] = np.dot(h, t1[t:t + LH])

    # right correction D [DBLK*128, 16]: s = (y1[L-8..L-1], u[L-8..L-1])
    NTAIL = DBLK * BLK
    D = np.zeros((NTAIL, 16))
    EXT = LH + 16
    for ib in range(16):
        y1t = np.zeros(NO)
        ut = np.zeros(NO)
        if ib < 8:
            y1t[ib] = 1.0
        else:
            ut[ib - 8] = 1.0
        yy = np.zeros(NO + EXT)
        uu = np.zeros(NO + EXT)
        yy[:NO] = y1t
        uu[:NO] = ut
        for t in range(NO, NO + EXT):
            v = 0.0
            for k in range(1, NO + 1):
                v -= ah[k] * yy[t - k]
            for k in range(0, NO + 1):
                if 0 <= t - k < NO:
                    v += bh[k] * uu[t - k]
            yy[t] = v
        ringout = yy[NO:]
        c = np.zeros(NTAIL)
        for idx in range(NTAIL):
            t_off = NTAIL - idx
            kk = np.arange(EXT)
            hidx = kk + t_off
            valid = hidx < LH
            c[idx] = -np.dot(h[hidx[valid]], ringout[valid])
        if ib == 7:                          # zi2 transient, scaled by y1[L-1]
            tr = ar_resp(np.concatenate([zi, np.zeros(NTAIL - NO)]))
            c += tr[NTAIL - 1 - np.arange(NTAIL)]
        D[:, ib] = c

    # Htail_c [8, 128]: y1last8[i] = sum_c Htail_c[i,:] @ u_{NB-1-c}
    HtailT = np.zeros((BLK, 3 * NO))
    for cblk in range(3):
        for i in range(NO):
            for p in range(BLK):
                k = (cblk + 1) * BLK - 1 - (7 - i) - p
                if 0 <= k < LH:
                    HtailT[p, NO * cblk + i] = h[k]

    out = {
        "WF": _round_f32r(np.concatenate(Fts, axis=1)),      # [128, NF*128]
        "HT": HtailT.astype(np.float32),                     # [128, 24]
        "DT": np.concatenate(
            [D[jb * BLK:(jb + 1) * BLK].T for jb in range(DBLK)],
            axis=1).astype(np.float32),                      # [16, DBLK*128]
        "WL": _round_f32r(wl.reshape(1, WLB * BLK)),         # [1, WLB*128]
    }
    _matrix_cache[key] = out
    return out


# ---------------- device kernel ----------------
def _gen_nc():
    nc = bacc.Bacc(None, target_bir_lowering=False)
    blob = nc.dram_tensor("blob", [128, BLOBC], F32R, kind="ExternalInput")
    yout = nc.dram_tensor("y", [128, CR], ODT, kind="ExternalOutput")

    with tile.TileContext(nc) as tc:
        with (
            tc.tile_pool(name="data", bufs=1) as dp,
            tc.tile_pool(name="psum", bufs=7, space="PSUM") as pp,
            tc.tile_pool(name="psumc", bufs=1, space="PSUM") as pc,
        ):
            WF = dp.tile([128, NF * BLK], F32R, tag="WF")
            UH = dp.tile([128, 36], F32, tag="UH")      # U3 | HT
            DS = dp.tile([16, DS_COLS], F32, tag="DS")  # D lhsT | s
            WX = dp.tile([1, WX_COLS], F32R, tag="WX")  # wl lhsT | x0
            U = dp.tile([128, UCOLS], F32R, tag="U")
            Y2 = dp.tile([128, CR], ODT, tag="Y2")
            U3 = UH[:, 0:12]
            HT = UH[:, 12:36]
            Svec = DS[:, DBLK * BLK:DBLK * BLK + LPC]

            # small/weight DMAs on sync(SP, HWDGE); U strips split over
            # gpsimd(SWDGE) and vector(HWDGE), in processing order
            nc.sync.dma_start(UH[:], blob[:, UH_OFF:UH_OFF + 36].bitcast(F32))
            nc.sync.dma_start(DS[:], blob[0:16, D_OFF:D_OFF + DS_COLS]
                              .bitcast(F32))
            nc.sync.dma_start(WX[:], blob[0:1, WX_OFF:WX_OFF + WX_COLS])
            nc.sync.dma_start(WF[:], blob[:, WF_OFF:WF_OFF + NF * BLK])
            ORDER = [9, 0, 8, 1, 7, 2, 6, 3, 5, 4]
            for i, k in enumerate(ORDER):
                o0 = SW * k
                o1 = min(SW * (k + 1) + 16, UCOLS)
                eng = nc.gpsimd if i % 2 == 0 else nc.scalar
                eng.dma_start(U[:, o0:o1], blob[:, UOFF + o0:UOFF + o1])

            # prelude (all tiny): y1-tail matmuls; left-zi outer products
            aux = pc.tile([128, 24], F32, tag="aux")
            psv = aux[0:NO, 8:8 + LPC]
            pw = aux[:, 0:NO]
            pd = aux[:, 12:24]
            for cblk in range(3):
                nc.tensor.matmul(psv, HT[:, NO * cblk:NO * (cblk + 1)],
                                 U3[:, (2 - cblk) * LPC:(3 - cblk) * LPC],
                                 start=(cblk == 0), stop=(cblk == 2))
            nc.vector.tensor_copy(Svec[0:NO, :], psv)
            for bwl in range(WLB):
                nc.tensor.matmul(pw[:, LPC * bwl:LPC * (bwl + 1)],
                                 WX[0:1, BLK * bwl:BLK * (bwl + 1)],
                                 WX[0:1, WLB * BLK:WLB * BLK + LPC],
                                 start=True, stop=True)

            # out-dma pair shipped after processing step i (column range)
            SHIP = {2: (8, 10, nc.sync), 3: (0, 2, nc.gpsimd),
                    6: (6, 8, nc.sync), 7: (2, 4, nc.gpsimd),
                    9: (4, 6, nc.sync)}
            for i, k in enumerate(ORDER):
                c0, c1 = SW * k, SW * (k + 1)
                u0 = PF + c0
                pm = pp.tile([128, SW], F32, tag="pm")
                for idx in range(NF):
                    j = (0, -1, 1, -2, 2)[idx]
                    nc.tensor.matmul(
                        pm[:], WF[:, BLK * (j + JUSE):BLK * (j + JUSE + 1)],
                        U[:, u0 + 4 * j:u0 + SW + 4 * j],
                        start=(idx == 0), stop=(idx == NF - 1))
                if i % 2 == 0:
                    nc.vector.tensor_copy(Y2[:, c0:c1], pm[:])
                else:
                    nc.scalar.copy(Y2[:, c0:c1], pm[:])

                if k == NS - 1:
                    # D-path: right-edge matmuls after strip 9 on the PE queue
                    for jb in range(DBLK):
                        nc.tensor.matmul(pd[:, LPC * jb:LPC * (jb + 1)],
                                         DS[:, BLK * jb:BLK * (jb + 1)],
                                         Svec, start=True, stop=True)
                    nc.vector.tensor_add(Y2[:, CR - DBLK * LPC:CR],
                                         Y2[:, CR - DBLK * LPC:CR], pd)
                if k == 0:
                    nc.vector.tensor_add(Y2[:, 0:WLB * LPC],
                                         Y2[:, 0:WLB * LPC], pw)
                if i in SHIP:
                    s0, s1, eng = SHIP[i]
                    eng.dma_start(yout[:, SW * s0:SW * s1],
                                  Y2[:, SW * s0:SW * s1])
    nc.compile()
    return nc


def _get_nc():
    if "nc" not in _nc_cache:
        _nc_cache["nc"] = _gen_nc()
    return _nc_cache["nc"]


def _bf16_to_f32(arr):
    a = np.asarray(arr)
    if a.dtype == np.float32:
        return a
    u = a.view(np.uint16).astype(np.uint32) << 16
    return u.view(np.float32)


# ---------------- host orchestration ----------------
def kernel(x, b=None, a=None):
    global last_exec_time_ns
    x = np.asarray(x)
    in_dtype = x.dtype
    if b is None or a is None:
        raise ValueError("need filter coefficients")
    b64 = np.asarray(b, dtype=np.float64)
    a64 = np.asarray(a, dtype=np.float64)
    W = _build_matrices(b64, a64)

    xl = np.asarray(x, dtype=np.float64).reshape(LANES, T)
    left = 2 * xl[:, :1] - xl[:, PADLEN:0:-1]
    right = 2 * xl[:, -1:] - xl[:, -2:-(PADLEN + 2):-1]
    ext = np.zeros((LANES, L), dtype=np.float32)
    ext[:, Z0:Z0 + PADLEN] = left
    ext[:, Z0 + PADLEN:Z0 + PADLEN + T] = xl
    ext[:, Z0 + PADLEN + T:] = right

    wcols = np.zeros((128, UOFF), dtype=np.float32)
    wcols[:, WF_OFF:WF_OFF + NF * BLK] = W["WF"]
    wcols[:, UH_OFF + 12:UH_OFF + 36] = W["HT"]
    wcols[0:16, D_OFF:D_OFF + DBLK * BLK] = W["DT"]
    wcols[0:1, WX_OFF:WX_OFF + WLB * BLK] = W["WL"]

    in_maps = []
    for core in range(N_CORES):
        lanes = ext[core * LPC:(core + 1) * LPC]             # [LPC, L]
        ublk = lanes.reshape(LPC, NB, BLK).transpose(2, 1, 0).reshape(128, CR)
        blob = np.zeros((128, BLOBC), dtype=np.float32)
        blob[:, :UOFF] = wcols
        blob[:, UH_OFF:UH_OFF + 12] = ublk[:, CR - 12:CR]    # unrounded tails
        blob[8:16, D_OFF + DBLK * BLK:D_OFF + DBLK * BLK + LPC] = (
            ublk[120:128, CR - LPC:CR])                      # u last-8 per lane
        blob[0:1, WX_OFF + WLB * BLK:WX_OFF + WLB * BLK + LPC] = (
            _round_f32r(lanes[:, Z0]))
        blob[:, UOFF + PF:UOFF + PF + CR] = _round_f32r(ublk)
        in_maps.append({"blob": blob})

    nc = _get_nc()
    trace = bool(int(os.environ.get("BASS_KERNEL_TRACE", "0")))
    res = run_bass_kernel_spmd(nc, in_maps, core_ids=list(range(N_CORES)),
                               trace=trace)
    last_exec_time_ns = res.exec_time_ns

    out = np.empty((LANES, T), dtype=np.float32)
    for core in range(N_CORES):
        ycore = _bf16_to_f32(res.results[core]["y"])         # [128, CR]
        lanes_y = (ycore.reshape(128, NB, LPC).transpose(2, 1, 0)
                   .reshape(LPC, L))
        out[core * LPC:(core + 1) * LPC] = (
            lanes_y[:, Z0 + PADLEN:Z0 + PADLEN + T])
    return out.reshape(BSH, CSH, T).astype(in_dtype)


# revision 8
# speedup vs baseline: 1.6207x; 1.6207x over previous
"""Trainium2 Bass kernel for zero-phase Butterworth band-stop filter (filtfilt).

Single fused pass: both filtfilt IIR sweeps collapse into one banded
block-Toeplitz convolution with the symmetric autocorrelation kernel
g = h (*) h_rev of the filter impulse response h:

    y[m] = sum_{j=-J..J} F_j @ u[m+j]    (F_j[i,p] = g[i - p - 128 j])

plus two small boundary terms (all matrices host-built in float64):
  * left:  zi transient of pass 1, rank-1 per lane in x0 = ext[Z0]
           (outer-product matmuls with contraction dim 1)
  * right: pass-2 right-edge correction D @ s, where s is the 16-dim
           state (last-8 y1, last-8 u); computed in full fp32 because D
           has ~70x non-normal cancellation. y1's last 8 samples come
           from 3 small fp32 matmuls against unrounded input tails.

All full-width matmuls run in float32r (host-prerounded inputs), which
streams at 1 column/cycle on the PE instead of fp32's 4. Data layout is
block-major ([128 rows = in-block position] x [col = block*4 + lane])
with 8 zero-pad columns on each side, so the shifted operands of F_j are
plain column-offset views of one SBUF tile - no stack DMAs at all.
Output is shipped as bf16 (upcast on host): rounding adds ~2e-3 relmax,
10% of the tolerance, and halves the output DMA bytes.

Sharding: 32 lanes (batch*channel), 4 per NeuronCore across 8 cores.
"""
import os

import numpy as np

import concourse.bacc as bacc
import concourse.mybir as mybir
import concourse.tile as tile
from concourse.bass_utils import run_bass_kernel_spmd

# ---------------- problem geometry (hardcoded for this problem) ----------------
BSH, CSH, T = 4, 8, 131072
LANES = BSH * CSH               # 32
N_CORES = 8
LPC = LANES // N_CORES          # 4 lanes per core
PADLEN = 27
BLK = 128
Z0 = 74                          # front zero padding so ext ends on block edge
L = Z0 + T + 2 * PADLEN          # 131200 samples per lane
NB = L // BLK                    # 1025 blocks per lane
CR = LPC * NB                    # 4100 real columns per core
PF = 8                           # front zero-pad cols (2 blocks)
PB = 8                           # back zero-pad cols
UCOLS = PF + CR + PB             # 4116
NS = 10                          # matmul strips
SW = CR // NS                    # 410 cols per strip (psum bank = 512 f32 max)
JUSE = 1                         # F_j for j in [-JUSE, JUSE]
NF = 2 * JUSE + 1
LH = 640                         # impulse-response length kept
WLB = 2                          # left-zi blocks corrected
DBLK = 3                         # right-edge blocks corrected
NO = 8                           # filter order
OUT_BF16 = True

F32 = mybir.dt.float32
F32R = mybir.dt.float32r
BF16 = mybir.dt.bfloat16
ODT = BF16 if OUT_BF16 else F32

# blob column layout (f32r dram [128, BLOBC])
WF_OFF = 0                       # F lhsT   [128, NF*128] f32r
UH_OFF = WF_OFF + NF * BLK       # U3 [128,12] + HT [128,24] f32 (bitcast)
D_OFF = UH_OFF + 36              # D lhsT rows 0:16 [16, DBLK*128] f32
#   + utail at rows 8:16, cols D_OFF+DBLK*128 : +4 (f32)
DS_COLS = DBLK * BLK + 4
WX_OFF = D_OFF + DS_COLS         # wl lhsT rows 0:1 [1, WLB*128] + x0 [1,4] f32r
WX_COLS = WLB * BLK + 4
UOFF = WX_OFF + WX_COLS          # U region [128, UCOLS] f32r (pads zeroed)
BLOBC = UOFF + UCOLS

_matrix_cache: dict = {}
_nc_cache: dict = {}
last_exec_time_ns = None


# ---------------- host-side matrix construction (float64) ----------------
def _round_f32r(v):
    """fp32r pre-rounding: bf16 hi + bf16 lo split (matches device cast)."""
    def bf16(x):
        u = np.ascontiguousarray(x, dtype=np.float32).view(np.uint32)
        return (((u + 0x7FFF + ((u >> 16) & 1)) & 0xFFFF0000)
                .astype(np.uint32)).view(np.float32)
    v32 = np.asarray(v, dtype=np.float32)
    hi = bf16(v32)
    lo = bf16((v32.astype(np.float64) - hi.astype(np.float64)).astype(np.float32))
    return (hi.astype(np.float64) + lo.astype(np.float64)).astype(np.float32)


def _build_matrices(b64, a64):
    key = (b64.tobytes(), a64.tobytes())
    if key in _matrix_cache:
        return _matrix_cache[key]
    bh = b64 / a64[0]
    ah = a64 / a64[0]

    def lfilter1(x):
        y = np.empty_like(x)
        z = np.zeros(NO)
        for t in range(x.shape[0]):
            xt = x[t]
            yt = bh[0] * xt + z[0]
            z[:-1] = z[1:]
            z[-1] = 0.0
            z += bh[1:] * xt - ah[1:] * yt
            y[t] = yt
        return y

    def ar_resp(drive):
        y = np.zeros(drive.shape[0])
        for t in range(y.shape[0]):
            v = drive[t]
            for k in range(1, NO + 1):
                if t - k >= 0:
                    v -= ah[k] * y[t - k]
            y[t] = v
        return y

    imp = np.zeros(LH)
    imp[0] = 1.0
    h = lfilter1(imp)
    g = np.correlate(h, h, mode="full")
    g0 = LH - 1

    ii = np.arange(BLK)[:, None]
    pp = np.arange(BLK)[None, :]
    Fts = []
    for j in range(-JUSE, JUSE + 1):
        d = ii - pp - BLK * j
        Fj = np.zeros((BLK, BLK))
        mask = np.abs(d) <= (LH - 1)
        Fj[mask] = g[d[mask] + g0]
        Fts.append(Fj.T.copy())

    A = np.zeros((NO, NO))
    A[0] = -ah[1:]
    A[np.arange(1, NO), np.arange(0, NO - 1)] = 1.0
    zi = np.linalg.solve(np.eye(NO) - A.T, bh[1:] - ah[1:] * bh[0])

    # left correction: zi transient of pass 1 through anticausal pass 2
    LT = WLB * BLK
    drive = np.zeros(LT + LH)
    drive[Z0:Z0 + NO] = zi
    t1 = ar_resp(drive)
    wl = np.zeros(LT)
    for t in range(LT):
        wl[t] = np.dot(h, t1[t:t + LH])

    # right correction D [DBLK*128, 16]: s = (y1[L-8..L-1], u[L-8..L-1])
    NTAIL = DBLK * BLK
    D = np.zeros((NTAIL, 16))
    EXT = LH + 16
    for ib in range(16):
        y1t = np.zeros(NO)
        ut = np.zeros(NO)
        if ib < 8:
            y1t[ib] = 1.0
        else:
            ut[ib - 8] = 1.0
        yy = np.zeros(NO + EXT)
        uu = np.zeros(NO + EXT)
        yy[:NO] = y1t
        uu[:NO] = ut
        for t in range(NO, NO + EXT):
            v = 0.0
            for k in range(1, NO + 1):
                v -= ah[k] * yy[t - k]
            for k in range(0, NO + 1):
                if 0 <= t - k < NO:
                    v += bh[k] * uu[t - k]
            yy[t] = v
        ringout = yy[NO:]
        c = np.zeros(NTAIL)
        for idx in range(NTAIL):
            t_off = NTAIL - idx
            kk = np.arange(EXT)
            hidx = kk + t_off
            valid = hidx < LH
            c[idx] = -np.dot(h[hidx[valid]], ringout[valid])
        if ib == 7:                          # zi2 transient, scaled by y1[L-1]
            tr = ar_resp(np.concatenate([zi, np.zeros(NTAIL - NO)]))
            c += tr[NTAIL - 1 - np.arange(NTAIL)]
        D[:, ib] = c

    # Htail_c [8, 128]: y1last8[i] = sum_c Htail_c[i,:] @ u_{NB-1-c}
    HtailT = np.zeros((BLK, 3 * NO))
    for cblk in range(3):
        for i in range(NO):
            for p in range(BLK):
                k = (cblk + 1) * BLK - 1 - (7 - i) - p
                if 0 <= k < LH:
                    HtailT[p, NO * cblk + i] = h[k]

    out = {
        "WF": _round_f32r(np.concatenate(Fts, axis=1)),      # [128, NF*128]
        "HT": HtailT.astype(np.float32),                     # [128, 24]
        "DT": np.concatenate(
            [D[jb * BLK:(jb + 1) * BLK].T for jb in range(DBLK)],
            axis=1).astype(np.float32),                      # [16, DBLK*128]
        "WL": _round_f32r(wl.reshape(1, WLB * BLK)),         # [1, WLB*128]
    }
    _matrix_cache[key] = out
    return out


# ---------------- device kernel ----------------
def _gen_nc():
    nc = bacc.Bacc(None, target_bir_lowering=False)
    blob = nc.dram_tensor("blob", [128, BLOBC], F32R, kind="ExternalInput")
    yout = nc.dram_tensor("y", [128, CR], ODT, kind="ExternalOutput")

    with tile.TileContext(nc) as tc:
        with (
            tc.tile_pool(name="data", bufs=1) as dp,
            tc.tile_pool(name="psum", bufs=7, space="PSUM") as pp,
            tc.tile_pool(name="psumc", bufs=1, space="PSUM") as pc,
        ):
            WF = dp.tile([128, NF * BLK], F32R, tag="WF")
            UH = dp.tile([128, 36], F32, tag="UH")      # U3 | HT
            DS = dp.tile([16, DS_COLS], F32, tag="DS")  # D lhsT | s
            WX = dp.tile([1, WX_COLS], F32R, tag="WX")  # wl lhsT | x0
            SEG = SW + 16                                # private strip segment
            U = dp.tile([128, NS * SEG], F32R, tag="U")
            Y2 = dp.tile([128, CR], ODT, tag="Y2")
            U3 = UH[:, 0:12]
            HT = UH[:, 12:36]
            Svec = DS[:, DBLK * BLK:DBLK * BLK + LPC]

            # weight/small DMAs on sync(SP, HWDGE); U segments split over
            # gpsimd(SWDGE) and scalar(HWDGE), in processing order. Each strip
            # gets a private segment (apron cols re-read from DRAM) so no two
            # input DMAs overlap in SBUF - overlap would chain them serially.
            nc.sync.dma_start(WF[:], blob[:, WF_OFF:WF_OFF + NF * BLK])
            nc.sync.dma_start(UH[:], blob[:, UH_OFF:UH_OFF + 36].bitcast(F32))
            nc.sync.dma_start(DS[:], blob[0:16, D_OFF:D_OFF + DS_COLS]
                              .bitcast(F32))
            nc.sync.dma_start(WX[:], blob[0:1, WX_OFF:WX_OFF + WX_COLS])
            ORDER = [9, 0, 8, 1, 7, 2, 6, 3, 5, 4]
            for i, k in enumerate(ORDER):
                o0 = SW * k
                eng = nc.gpsimd if i % 2 == 0 else nc.scalar
                eng.dma_start(U[:, SEG * k:SEG * k + SEG],
                              blob[:, UOFF + o0:UOFF + o0 + SEG])

            aux = pc.tile([128, 24], F32, tag="aux")
            psv = aux[0:NO, 8:8 + LPC]
            pw = aux[:, 0:NO]
            pd = aux[:, 12:24]

            # out-dma pair shipped after processing step i (column range)
            SHIP = {2: (8, 10, nc.sync), 3: (0, 2, nc.gpsimd),
                    6: (6, 8, nc.sync), 7: (2, 4, nc.gpsimd),
                    9: (4, 6, nc.sync)}
            for i, k in enumerate(ORDER):
                c0, c1 = SW * k, SW * (k + 1)
                u0 = PF + c0
                pm = pp.tile([128, SW], F32, tag="pm")
                ub = SEG * k + 8                     # local col of strip start
                for idx in range(NF):
                    j = (0, -1, 1, -2, 2)[idx]
                    nc.tensor.matmul(
                        pm[:], WF[:, BLK * (j + JUSE):BLK * (j + JUSE + 1)],
                        U[:, ub + 4 * j:ub + SW + 4 * j],
                        start=(idx == 0), stop=(idx == NF - 1))
                if i % 2 == 0:
                    nc.vector.tensor_copy(Y2[:, c0:c1], pm[:])
                else:
                    nc.scalar.copy(Y2[:, c0:c1], pm[:])

                if k == NS - 1:
                    # edge paths (all tiny), tucked behind strip 9 on PE
                    for cblk in range(3):
                        nc.tensor.matmul(
                            psv, HT[:, NO * cblk:NO * (cblk + 1)],
                            U3[:, (2 - cblk) * LPC:(3 - cblk) * LPC],
                            start=(cblk == 0), stop=(cblk == 2))
                    nc.vector.tensor_copy(Svec[0:NO, :], psv)
                    for bwl in range(WLB):
                        nc.tensor.matmul(pw[:, LPC * bwl:LPC * (bwl + 1)],
                                         WX[0:1, BLK * bwl:BLK * (bwl + 1)],
                                         WX[0:1, WLB * BLK:WLB * BLK + LPC],
                                         start=True, stop=True)
                    for jb in range(DBLK):
                        nc.tensor.matmul(pd[:, LPC * jb:LPC * (jb + 1)],
                                         DS[:, BLK * jb:BLK * (jb + 1)],
                                         Svec, start=True, stop=True)
                    nc.vector.tensor_add(Y2[:, CR - DBLK * LPC:CR],
                                         Y2[:, CR - DBLK * LPC:CR], pd)
                if k == 0:
                    nc.vector.tensor_add(Y2[:, 0:WLB * LPC],
                                         Y2[:, 0:WLB * LPC], pw)
                if i in SHIP:
                    s0, s1, eng = SHIP[i]
                    eng.dma_start(yout[:, SW * s0:SW * s1],
                                  Y2[:, SW * s0:SW * s1])
    nc.compile()
    return nc


def _get_nc():
    if "nc" not in _nc_cache:
        _nc_cache["nc"] = _gen_nc()
    return _nc_cache["nc"]


def _bf16_to_f32(arr):
    a = np.asarray(arr)
    if a.dtype == np.float32:
        return a
    u = a.view(np.uint16).astype(np.uint32) << 16
    return u.view(np.float32)


# ---------------- host orchestration ----------------
def kernel(x, b=None, a=None):
    global last_exec_time_ns
    x = np.asarray(x)
    in_dtype = x.dtype
    if b is None or a is None:
        raise ValueError("need filter coefficients")
    b64 = np.asarray(b, dtype=np.float64)
    a64 = np.asarray(a, dtype=np.float64)
    W = _build_matrices(b64, a64)

    xl = np.asarray(x, dtype=np.float64).reshape(LANES, T)
    left = 2 * xl[:, :1] - xl[:, PADLEN:0:-1]
    right = 2 * xl[:, -1:] - xl[:, -2:-(PADLEN + 2):-1]
    ext = np.zeros((LANES, L), dtype=np.float32)
    ext[:, Z0:Z0 + PADLEN] = left
    ext[:, Z0 + PADLEN:Z0 + PADLEN + T] = xl
    ext[:, Z0 + PADLEN + T:] = right

    wcols = np.zeros((128, UOFF), dtype=np.float32)
    wcols[:, WF_OFF:WF_OFF + NF * BLK] = W["WF"]
    wcols[:, UH_OFF + 12:UH_OFF + 36] = W["HT"]
    wcols[0:16, D_OFF:D_OFF + DBLK * BLK] = W["DT"]
    wcols[0:1, WX_OFF:WX_OFF + WLB * BLK] = W["WL"]

    in_maps = []
    for core in range(N_CORES):
        lanes = ext[core * LPC:(core + 1) * LPC]             # [LPC, L]
        ublk = lanes.reshape(LPC, NB, BLK).transpose(2, 1, 0).reshape(128, CR)
        blob = np.zeros((128, BLOBC), dtype=np.float32)
        blob[:, :UOFF] = wcols
        blob[:, UH_OFF:UH_OFF + 12] = ublk[:, CR - 12:CR]    # unrounded tails
        blob[8:16, D_OFF + DBLK * BLK:D_OFF + DBLK * BLK + LPC] = (
            ublk[120:128, CR - LPC:CR])                      # u last-8 per lane
        blob[0:1, WX_OFF + WLB * BLK:WX_OFF + WLB * BLK + LPC] = (
            _round_f32r(lanes[:, Z0]))
        blob[:, UOFF + PF:UOFF + PF + CR] = _round_f32r(ublk)
        in_maps.append({"blob": blob})

    nc = _get_nc()
    trace = bool(int(os.environ.get("BASS_KERNEL_TRACE", "0")))
    res = run_bass_kernel_spmd(nc, in_maps, core_ids=list(range(N_CORES)),
                               trace=trace)
    last_exec_time_ns = res.exec_time_ns

    out = np.empty((LANES, T), dtype=np.float32)
    for core in range(N_CORES):
        ycore = _bf16_to_f32(res.results[core]["y"])         # [128, CR]
        lanes_y = (ycore.reshape(128, NB, LPC).transpose(2, 1, 0)
                   .reshape(LPC, L))
        out[core * LPC:(core + 1) * LPC] = (
            lanes_y[:, Z0 + PADLEN:Z0 + PADLEN + T])
    return out.reshape(BSH, CSH, T).astype(in_dtype)
